# revision 23
# baseline (speedup 1.0000x reference)
"""AFT-Full distributed Trainium2 kernel (v5).

Reference computation (B=8, T=4096, D=512, H=64):
    Q = x @ wq.T ; K = x @ wk.T ; V = x @ wv.T                     [B,T,H]
    ew  = exp(wbias)                                               [T,T]
    num = ew @ (exp(K)*V) ; den = ew @ exp(K)                      [B,T,H]
    out = (sigmoid(Q) * num/den) @ wp.T + bp                       [B,T,D]

Sharding: 4 batch-groups x 2 t-groups (core c: batches {2*(c//2),
2*(c//2)+1}, t-slice c%2).  No collectives.

v5 vs the 114us v4 (SwInterleave):
- x ships in 512KB half-tiles consumed at (batch, 512-s-block)
  granularity, so phase 1 starts ~1.4us after launch and never waits a
  full 1MB tile.
- all of phase 1 + Q + colsum run first (PE-paced inside the x-DMA
  window) while r streams behind x; phase 2 then runs as one stream
  over chunk-pairs with THREE t-passes live (pass 2 reuses the freed
  colsum PSUM banks), matching PE consumption to r arrival with
  SBUF-buffered slack.  Pass 3 re-reads resident r tiles and is
  spliced with the phase-3 epilogues of passes 0-2.
- phase-2 matmuls are fp8e4m3 DoubleRowSwInterleave (256-deep
  contraction at 2x rate); the fp8 z copy is written directly in the
  interleaved weights layout by strided DVE casts.  The plain
  DoubleRow perf mode NaNs on HW through this toolchain; SwInterleave
  (software-interleaved weights) works.  The interleave's inherent
  column reversal makes nd partitions come out h-reversed: absorbed by
  h-reversed wq/bq/wp.T host packing and an on-chip csv reversal via
  one f32 matmul against a host-shipped anti-identity.
- exact colsum (the bulk of num/den) still reads bf16 z, so fp8 error
  only touches the small ew-1 correction term.
"""

import sys

for _p in ("/opt/trn_rl_repo", "/opt/pypackages"):
    if _p not in sys.path:
        sys.path.append(_p)

import numpy as np
import ml_dtypes

B, T, D, H = 8, 4096, 512, 64
BG, TG = 4, 2            # batch groups x t groups = 8 cores
BPC = B // BG            # batches per core
TPC = T // TG            # t rows per core
NH = 8                   # x s-halves of 512
NCP = 16                 # r chunk-pairs of 256 s rows
NP = 4                   # phase-2/3 t-passes of 512
TP = TPC // NP
SCALE = 64.0
N_CORES = 8

_NC_CACHE = {}


def _build_module(use_bias):
    import concourse.bass as bass
    import concourse.mybir as mybir
    import concourse.tile as tile
    from concourse import bacc
    from contextlib import ExitStack

    bf16 = mybir.dt.bfloat16
    f32 = mybir.dt.float32
    f8 = mybir.dt.float8e4
    DR = mybir.MatmulPerfMode.DoubleRowSwInterleave
    Exp = mybir.ActivationFunctionType.Exp
    Sigmoid = mybir.ActivationFunctionType.Sigmoid
    Identity = mybir.ActivationFunctionType.Identity
    mult = mybir.AluOpType.mult

    nc = bacc.Bacc("TRN2", target_bir_lowering=False, debug=False,
                   num_devices=N_CORES)

    xp = nc.dram_tensor("xp", [BPC, NH, 128, 2048], bf16,
                        kind="ExternalInput").ap()
    rp = nc.dram_tensor("rp", [NCP, 128, 2 * TPC], f8,
                        kind="ExternalInput").ap()
    wkv = nc.dram_tensor("wkv", [128, 4 * 2 * H], bf16,
                         kind="ExternalInput").ap()
    wqT = nc.dram_tensor("wqT", [128, 4 * H], bf16,
                         kind="ExternalInput").ap()
    wpT = nc.dram_tensor("wpT", [H + 1, D], bf16, kind="ExternalInput").ap()
    bkv = nc.dram_tensor("bkv", [1, 2 * H], bf16, kind="ExternalInput").ap()
    bqv = nc.dram_tensor("bqv", [H, 1], f32, kind="ExternalInput").ap()
    ones = nc.dram_tensor("ones", [1, 128], bf16, kind="ExternalInput").ap()
    antiI = nc.dram_tensor("antiI", [128, 128], f32,
                           kind="ExternalInput").ap()
    out = nc.dram_tensor("out", [BPC, TPC, D], bf16,
                         kind="ExternalOutput").ap()

    with tile.TileContext(nc) as tc, ExitStack() as ctx:
        wpool = ctx.enter_context(tc.tile_pool(name="wts", bufs=1))
        xpool = ctx.enter_context(tc.tile_pool(name="xg", bufs=2 * NH))
        rpool = ctx.enter_context(tc.tile_pool(name="rr", bufs=NCP))
        zpool = ctx.enter_context(tc.tile_pool(name="z", bufs=BPC))
        zqpool = ctx.enter_context(tc.tile_pool(name="zq", bufs=BPC))
        sqpool = ctx.enter_context(tc.tile_pool(name="sq", bufs=BPC))
        cpool = ctx.enter_context(tc.tile_pool(name="csv", bufs=1))
        ytpool = ctx.enter_context(tc.tile_pool(name="yt", bufs=3))
        tpool = ctx.enter_context(tc.tile_pool(name="tmp", bufs=2))
        opool = ctx.enter_context(tc.tile_pool(name="osb", bufs=4))
        pps = ctx.enter_context(tc.tile_pool(name="pps", bufs=2,
                                             space="PSUM"))
        pnd = ctx.enter_context(tc.tile_pool(name="pnd", bufs=4,
                                             space="PSUM"))
        pcs = ctx.enter_context(tc.tile_pool(name="pcs", bufs=2,
                                             space="PSUM"))

        # --- resident weights / constants on the scalar queue ---
        wkv_sb = wpool.tile([128, 4 * 2 * H], bf16)     # [128, 512]
        nc.scalar.dma_start(wkv_sb[:, :], wkv[:, :])
        wq_sb = wpool.tile([128, 4 * H], bf16)          # [128, 256]
        nc.scalar.dma_start(wq_sb[:, :], wqT[:, :])
        wp_sb = wpool.tile([H + 1, D], bf16)            # [65, 512]
        nc.scalar.dma_start(wp_sb[:, :], wpT[:, :])
        bkv_sb = wpool.tile([1, 2 * H], bf16)
        nc.scalar.dma_start(bkv_sb[:, :], bkv[:, :])
        bq_sb = wpool.tile([H, 1], f32)
        nc.scalar.dma_start(bq_sb[:, :], bqv[:, :])
        ones_sb = wpool.tile([1, 128], bf16)
        nc.scalar.dma_start(ones_sb[:, :], ones[:, :])
        antiI_sb = wpool.tile([128, 128], f32, name="antiI")
        nc.scalar.dma_start(antiI_sb[:, :], antiI[:, :])
        col64 = wpool.tile([128, 1], bf16, name="col64")
        nc.gpsimd.memset(col64[:, :], SCALE)
        one1 = wpool.tile([1, 1], f32, name="one1")
        nc.gpsimd.memset(one1[:, :], 1.0)
        warm = wpool.tile([128, 512], bf16, name="warm")
        nc.gpsimd.memset(warm[:, :], 1.0)

        # --- input streams: x halves in consumption order on sync; r on
        # gpsimd, held back behind the first 4MB of x ---
        from concourse.tile import add_dep_helper
        xh = [[None] * NH for _ in range(BPC)]
        rr = [None] * NCP
        x_dmas = []

        for j in range(NH):
            for b in range(BPC):
                t_ = xpool.tile([128, 2048], bf16, name=f"x{b}h{j}",
                                tag="xg")
                x_dmas.append(nc.sync.dma_start(t_[:, :], xp[b, j]))
                xh[b][j] = t_
        for cp in range(NCP):
            t_ = rpool.tile([128, 2 * TPC], f8, name=f"r{cp}", tag="rr")
            rd = nc.gpsimd.dma_start(t_[:, :], rp[cp])
            if cp == 0:
                add_dep_helper(rd.ins, x_dmas[5].ins,
                               reason="delay r stream behind first 3MB of x")
            rr[cp] = t_

        z_sb = [zpool.tile([128, 32 * 128], bf16, name=f"z{b}", tag="z")
                for b in range(BPC)]
        zq_sb = [zqpool.tile([128, 32 * 128], f8, name=f"zq{b}", tag="zq")
                 for b in range(BPC)]
        sq_sb = [sqpool.tile([H, TPC], f32, name=f"sq{b}", tag="sq")
                 for b in range(BPC)]
        # one full PSUM bank per batch: start_tensor_calc zeroes a 2KB
        # region, so interleaved accumulation groups must not share a bank
        cs_ps = [pcs.tile([128, 512], f32, name=f"cs{b}", tag="cs")
                 for b in range(BPC)]

        # --- phase 1 at (batch, 512-s-block) granularity: Z = [eKV | eK],
        # colsum via tiny z-as-weights matmuls, interleaved fp8 copy ---
        def hb(b, j):
            pkv = pps.tile([128, 512], f32, name="pkv", tag="ps")
            for si in range(4):
                for d in range(4):
                    nc.tensor.matmul(
                        pkv[:, si * 128:(si + 1) * 128],
                        lhsT=xh[b][j][:, d * 512 + si * 128:
                                      d * 512 + si * 128 + 128],
                        rhs=wkv_sb[:, d * 128:(d + 1) * 128],
                        start=(d == 0),
                        stop=(not use_bias and d == 3))
                if use_bias:
                    nc.tensor.matmul(
                        pkv[:, si * 128:(si + 1) * 128],
                        lhsT=ones_sb[:, :], rhs=bkv_sb[:, :],
                        start=False, stop=True)
            c0 = j * 512
            pk3 = pkv[:, :].rearrange("p (c k) -> p c k", c=4)
            zg3 = z_sb[b][:, c0:c0 + 512].rearrange("p (c k) -> p c k", c=4)
            nc.scalar.activation(
                zg3[:, :, H:2 * H], pk3[:, :, 0:H], Exp)
            nc.vector.tensor_tensor(
                zg3[:, :, 0:H], pk3[:, :, H:2 * H],
                zg3[:, :, H:2 * H], mult)
            # fp8 copy in DoubleRowSwInterleave weights layout:
            # zq[p, c0 + q*256 + 2*k + i] = z[p, c0 + (2q+i)*128 + k]
            zsrc = z_sb[b][:, c0:c0 + 512].rearrange(
                "p (q i k) -> p q i k", q=2, i=2)
            zdst = zq_sb[b][:, c0:c0 + 512].rearrange(
                "p (q k i) -> p q k i", q=2, k=128)
            for i in range(2):
                nc.vector.tensor_copy(zdst[:, :, :, i], zsrc[:, :, i, :])
            for si in range(4):
                sc = j * 4 + si
                nc.tensor.matmul(
                    cs_ps[b][:, 0:1],
                    lhsT=z_sb[b][:, sc * 128:(sc + 1) * 128],
                    rhs=col64[:, :],
                    start=(sc == 0), stop=(sc == 31),
                    skip_group_check=True)

        def q_block(b, j):
            pq = pps.tile([128, 512], f32, name="pq", tag="ps")
            for d in range(4):
                nc.tensor.matmul(
                    pq[0:H, :],
                    lhsT=wq_sb[:, d * H:(d + 1) * H],
                    rhs=xh[b][j][:, d * 512:(d + 1) * 512],
                    start=(d == 0), stop=(d == 3))
            nc.scalar.activation(
                sq_sb[b][:, j * 512:(j + 1) * 512], pq[0:H, :],
                Sigmoid, bias=bq_sb[:, :])

        # --- phase 2: nd[p][b] += zq_cp^T @ r_cp over all 16 chunk-pairs
        # (fp8e4m3 DoubleRowSwInterleave: 256-deep contraction, 2x rate) ---
        nd = [[None] * BPC for _ in range(NP)]

        def ph2_cp(cp, passes):
            rr3 = rr[cp][:, :].rearrange("p (two t) -> p two t", two=2)
            for b in range(BPC):
                zq2 = zq_sb[b][:, cp * 256:(cp + 1) * 256]
                for p in passes:
                    nc.tensor.matmul(
                        nd[p][b][:, :],
                        lhsT=zq2[:, :],
                        rhs=rr3[:, :, p * TP:(p + 1) * TP],
                        start=(cp == 0), stop=(cp == NCP - 1),
                        perf_mode=DR)

        # --- phase 3 --- (SwInterleave reverses nd partitions: partition m
        # holds natural column 127-m, so [0:H]=den-rev, [H:2H]=num-rev and
        # csv is reversed to match via the antiI matmul)
        csv = cpool.tile([128, BPC], f32, name="csv", tag="csv")
        csr = cpool.tile([128, BPC], f32, name="csr", tag="csr")
        tmps = {}

        add = mybir.AluOpType.add

        def ph3_reads(p, b):
            ncp = tpool.tile([H, TP], f32, name="ncp", tag="ncp")
            nc.vector.tensor_scalar(ncp[:, :], nd[p][b][H:2 * H, :],
                                    csv[H:2 * H, b:b + 1], None, add)
            dcp = tpool.tile([H, TP], f32, name="dcp", tag="dcp")
            nc.scalar.activation(dcp[:, :], nd[p][b][0:H, :], Identity,
                                 bias=csv[0:H, b:b + 1])
            rec = tpool.tile([H, TP], f32, name="rec", tag="rec")
            nc.vector.reciprocal_approx_fast(rec[:, :], dcp[:, :])
            tmp = tpool.tile([H, TP], f32, name="tmp", tag="tmp", bufs=8)
            nc.vector.tensor_tensor(tmp[:, :], ncp[:, :], rec[:, :], mult)
            tmps[(p, b)] = tmp

        def ph3_tail(p, b):
            yt = ytpool.tile([H + 1, TP], bf16, name="yt", tag="yt")
            nc.gpsimd.memset(yt[H:H + 1, :], 1.0)
            nc.gpsimd.tensor_tensor(
                yt[0:H, :], tmps[(p, b)][:, :],
                sq_sb[b][:, p * TP:(p + 1) * TP], mult)
            for c in range(4):
                po = pps.tile([128, 512], f32, name="po", tag="ps")
                nc.tensor.matmul(po[:, :], lhsT=yt[:, c * 128:(c + 1) * 128],
                                 rhs=wp_sb[:, :], start=True, stop=True)
                osb = opool.tile([128, 512], bf16, name="osb", tag="osb")
                eng_c = (nc.scalar.copy, nc.vector.tensor_copy,
                         nc.scalar.copy, nc.vector.tensor_copy)[c]
                eng_c(osb[:, :], po[:, :])
                t0 = p * TP + c * 128
                eng = (nc.gpsimd, nc.sync)[c % 2]
                eng.dma_start(out[b, t0:t0 + 128, :], osb[:, :])

        # --- emission schedule (PE order = program order per engine) ---
        # warmup runway: ramp the PE clock while the DMA engines start up
        warm_ps = pps.tile([128, 512], f32, name="warm_ps", tag="ps")
        for w in range(16):
            nc.tensor.matmul(warm_ps[:, :], lhsT=warm[:, 0:128],
                             rhs=warm[:, :], start=True, stop=True,
                             skip_group_check=True)
        # phase 1 + colsum, paced to the x half-tile arrival order
        for j in range(NH):
            hb(0, j)
            hb(1, j)
        # csv: copy colsums out of PSUM, reverse via antiI matmul
        for b in range(BPC):
            nc.scalar.copy(csr[:, b:b + 1], cs_ps[b][:, 0:1])
            pcs_r = pps.tile([128, 512], f32, name="pcsr", tag="ps")
            nc.tensor.matmul(pcs_r[:, 0:1], lhsT=antiI_sb[:, :],
                             rhs=csr[:, b:b + 1], start=True, stop=True)
            nc.scalar.copy(csv[:, b:b + 1], pcs_r[:, 0:1])
        # phase-2 stream: passes 0-2 live (pass 2 in the freed colsum
        # banks); Q blocks interleaved as anti-starvation filler while the
        # r tiles stream in
        for p in (0, 1):
            for b in range(BPC):
                nd[p][b] = pnd.tile([128, TP], f32, name=f"nd{p}{b}",
                                    tag="nd")
        for b in range(BPC):
            # same tag as the colsum tiles: rotates into their freed banks
            nd[2][b] = pcs.tile([128, 512], f32, name=f"nd2{b}", tag="cs")
        for cp in range(NCP):
            ph2_cp(cp, (0, 1, 2))
            if cp < 8:
                q_block(cp % 2, cp // 2)
        # pass 3 (resident r tiles, nd in the free pps banks), epilogues
        ph3_reads(0, 0)
        ph3_reads(0, 1)
        ph3_reads(1, 0)
        ph3_reads(1, 1)
        for b in range(BPC):
            nd[3][b] = pps.tile([128, 512], f32, name=f"nd3{b}", tag="ps")
        for cp in range(NCP):
            ph2_cp(cp, (3,))
        ph3_reads(2, 0)
        ph3_reads(2, 1)
        ph3_reads(3, 0)
        ph3_reads(3, 1)
        ph3_tail(0, 0)
        ph3_tail(0, 1)
        ph3_tail(1, 0)
        ph3_tail(1, 1)
        ph3_tail(2, 0)
        ph3_tail(2, 1)
        ph3_tail(3, 0)
        ph3_tail(3, 1)

    nc.compile()
    from concourse.bass_interp import get_hw_module
    nc.m = get_hw_module(nc.m)
    return nc


def _get_module(use_bias):
    key = ("nc", use_bias)
    if key not in _NC_CACHE:
        _NC_CACHE[key] = _build_module(use_bias)
    return _NC_CACHE[key]


def kernel(x, wq, bq, wk, bk, wv, bv, wp, bp, wbias):
    from concourse.bass_utils import run_bass_kernel_spmd

    bf16 = ml_dtypes.bfloat16
    f8 = ml_dtypes.float8_e4m3
    x = np.asarray(x, np.float32)
    wbias = np.asarray(wbias, np.float32)
    wq, wk, wv, wp = (np.asarray(a, np.float32) for a in (wq, wk, wv, wp))
    bq, bk, bv, bp = (np.asarray(a, np.float32) for a in (bq, bk, bv, bp))

    xT_full = np.ascontiguousarray(x.transpose(0, 2, 1)).astype(bf16)
    rq_full = (SCALE * (np.exp(wbias) - 1.0)).T     # [s, t] f32

    # pack the d-chunked SBUF layouts host-side
    wkv_h = np.ascontiguousarray(
        np.concatenate([wk.T, wv.T], axis=1).astype(bf16)
        .reshape(4, 128, 2 * H).transpose(1, 0, 2).reshape(128, 4 * 2 * H))
    # h-reversed Q/projection weights: the SwInterleave phase-2 matmul emits
    # nd with reversed partitions, so sq/tmp/yt all live in reversed-h order
    wqT_h = np.ascontiguousarray(
        wq.T[:, ::-1].astype(bf16).reshape(4, 128, H).transpose(1, 0, 2)
        .reshape(128, 4 * H))
    wpT_h = np.concatenate(
        [wp.T[::-1], np.asarray(bp, np.float32)[None, :]],
        axis=0).astype(bf16)
    bkv_h = np.concatenate([bk, bv])[None, :].astype(bf16)         # [1, 2H]
    bq_h = np.asarray(bq, np.float32)[::-1, None].copy()           # [H, 1]
    ones_h = np.ones((1, 128), dtype=bf16)
    antiI_h = np.eye(128, dtype=np.float32)[:, ::-1].copy()        # [128,128]
    use_bias = bool(np.any(bk) or np.any(bv))

    # Per t-group: s-permuted inputs (own t-slice rows first) so the SPMD
    # graph reads Q's x columns at [0:TPC] on every core.
    perm = {}
    for tj in range(TG):
        perm[tj] = np.concatenate([
            np.arange(tj * TPC, (tj + 1) * TPC),
            np.arange(0, tj * TPC),
            np.arange((tj + 1) * TPC, T)])

    # r chunk-pair layout for DoubleRow*: rp[cp][p, i*TPC + t] =
    # r[s = 256*cp + 128*i + p, t]
    rp_tj = {}
    for tj in range(TG):
        rq = rq_full[perm[tj]][:, tj * TPC:(tj + 1) * TPC].astype(f8)
        rp_tj[tj] = np.ascontiguousarray(
            rq.reshape(NCP, 2, 128, TPC).transpose(0, 2, 1, 3)
            .reshape(NCP, 128, 2 * TPC))

    # x half-tiles: xp[b, j, p, d*512 + s'] = xT[b, d*128+p, j*512+s']
    xp_c = {}
    for bi in range(BG):
        for tj in range(TG):
            xt = xT_full[bi * BPC:(bi + 1) * BPC][:, :, perm[tj]]
            xp_c[(bi, tj)] = np.ascontiguousarray(
                xt.reshape(BPC, 4, 128, NH, 512).transpose(0, 3, 2, 1, 4)
                .reshape(BPC, NH, 128, 2048))

    in_maps = []
    for c in range(N_CORES):
        bi, tj = c // TG, c % TG
        in_maps.append({
            "xp": xp_c[(bi, tj)],
            "rp": rp_tj[tj],
            "wkv": wkv_h, "wqT": wqT_h, "wpT": wpT_h,
            "bkv": bkv_h, "bqv": bq_h, "ones": ones_h, "antiI": antiI_h,
        })

    nc = _get_module(use_bias)
    res = run_bass_kernel_spmd(nc, in_maps, core_ids=list(range(N_CORES)))

    full = np.empty((B, T, D), dtype=np.float32)
    for c in range(N_CORES):
        bi, tj = c // TG, c % TG
        full[bi * BPC:(bi + 1) * BPC, tj * TPC:(tj + 1) * TPC, :] = \
            res.results[c]["out"].astype(np.float32)
    return full


# revision 24
# speedup vs baseline: 1.0578x; 1.0578x over previous
"""AFT-Full distributed Trainium2 kernel (v5).

Reference computation (B=8, T=4096, D=512, H=64):
    Q = x @ wq.T ; K = x @ wk.T ; V = x @ wv.T                     [B,T,H]
    ew  = exp(wbias)                                               [T,T]
    num = ew @ (exp(K)*V) ; den = ew @ exp(K)                      [B,T,H]
    out = (sigmoid(Q) * num/den) @ wp.T + bp                       [B,T,D]

Sharding: 4 batch-groups x 2 t-groups (core c: batches {2*(c//2),
2*(c//2)+1}, t-slice c%2).  No collectives.

v5 vs the 114us v4 (SwInterleave):
- x ships in 512KB half-tiles consumed at (batch, 512-s-block)
  granularity, so phase 1 starts ~1.4us after launch and never waits a
  full 1MB tile.
- all of phase 1 + Q + colsum run first (PE-paced inside the x-DMA
  window) while r streams behind x; phase 2 then runs as one stream
  over chunk-pairs with THREE t-passes live (pass 2 reuses the freed
  colsum PSUM banks), matching PE consumption to r arrival with
  SBUF-buffered slack.  Pass 3 re-reads resident r tiles and is
  spliced with the phase-3 epilogues of passes 0-2.
- phase-2 matmuls are fp8e4m3 DoubleRowSwInterleave (256-deep
  contraction at 2x rate); the fp8 z copy is written directly in the
  interleaved weights layout by strided DVE casts.  The plain
  DoubleRow perf mode NaNs on HW through this toolchain; SwInterleave
  (software-interleaved weights) works.  The interleave's inherent
  column reversal makes nd partitions come out h-reversed: absorbed by
  h-reversed wq/bq/wp.T host packing and an on-chip csv reversal via
  one f32 matmul against a host-shipped anti-identity.
- exact colsum (the bulk of num/den) still reads bf16 z, so fp8 error
  only touches the small ew-1 correction term.
"""

import sys

for _p in ("/opt/trn_rl_repo", "/opt/pypackages"):
    if _p not in sys.path:
        sys.path.append(_p)

import numpy as np
import ml_dtypes

B, T, D, H = 8, 4096, 512, 64
BG, TG = 4, 2            # batch groups x t groups = 8 cores
BPC = B // BG            # batches per core
TPC = T // TG            # t rows per core
NH = 8                   # x s-halves of 512
NCP = 16                 # r chunk-pairs of 256 s rows
NP = 4                   # phase-2/3 t-passes of 512
TP = TPC // NP
SCALE = 64.0
N_CORES = 8

_NC_CACHE = {}


def _build_module(use_bias):
    import concourse.bass as bass
    import concourse.mybir as mybir
    import concourse.tile as tile
    from concourse import bacc
    from contextlib import ExitStack

    bf16 = mybir.dt.bfloat16
    f32 = mybir.dt.float32
    f8 = mybir.dt.float8e4
    DR = mybir.MatmulPerfMode.DoubleRowSwInterleave
    Exp = mybir.ActivationFunctionType.Exp
    Sigmoid = mybir.ActivationFunctionType.Sigmoid
    Identity = mybir.ActivationFunctionType.Identity
    mult = mybir.AluOpType.mult

    nc = bacc.Bacc("TRN2", target_bir_lowering=False, debug=False,
                   num_devices=N_CORES)

    xp = nc.dram_tensor("xp", [BPC, NH, 128, 2048], bf16,
                        kind="ExternalInput").ap()
    rp = nc.dram_tensor("rp", [NCP, 128, 2 * TPC], f8,
                        kind="ExternalInput").ap()
    wkv = nc.dram_tensor("wkv", [128, 4 * 2 * H], bf16,
                         kind="ExternalInput").ap()
    wqT = nc.dram_tensor("wqT", [128, 4 * H], bf16,
                         kind="ExternalInput").ap()
    wpT = nc.dram_tensor("wpT", [H + 1, D], bf16, kind="ExternalInput").ap()
    bkv = nc.dram_tensor("bkv", [1, 2 * H], bf16, kind="ExternalInput").ap()
    bqv = nc.dram_tensor("bqv", [H, 1], f32, kind="ExternalInput").ap()
    ones = nc.dram_tensor("ones", [1, 128], bf16, kind="ExternalInput").ap()
    antiI = nc.dram_tensor("antiI", [128, 128], f32,
                           kind="ExternalInput").ap()
    out = nc.dram_tensor("out", [BPC, TPC, D], bf16,
                         kind="ExternalOutput").ap()

    with tile.TileContext(nc) as tc, ExitStack() as ctx:
        wpool = ctx.enter_context(tc.tile_pool(name="wts", bufs=1))
        xpool = ctx.enter_context(tc.tile_pool(name="xg", bufs=2 * NH))
        rpool = ctx.enter_context(tc.tile_pool(name="rr", bufs=NCP))
        zpool = ctx.enter_context(tc.tile_pool(name="z", bufs=BPC))
        zqpool = ctx.enter_context(tc.tile_pool(name="zq", bufs=BPC))
        sqpool = ctx.enter_context(tc.tile_pool(name="sq", bufs=BPC))
        cpool = ctx.enter_context(tc.tile_pool(name="csv", bufs=1))
        ytpool = ctx.enter_context(tc.tile_pool(name="yt", bufs=3))
        tpool = ctx.enter_context(tc.tile_pool(name="tmp", bufs=2))
        opool = ctx.enter_context(tc.tile_pool(name="osb", bufs=4))
        pps = ctx.enter_context(tc.tile_pool(name="pps", bufs=2,
                                             space="PSUM"))
        pnd = ctx.enter_context(tc.tile_pool(name="pnd", bufs=4,
                                             space="PSUM"))
        pcs = ctx.enter_context(tc.tile_pool(name="pcs", bufs=2,
                                             space="PSUM"))

        # --- resident weights / constants on the scalar queue ---
        wkv_sb = wpool.tile([128, 4 * 2 * H], bf16)     # [128, 512]
        nc.scalar.dma_start(wkv_sb[:, :], wkv[:, :])
        wq_sb = wpool.tile([128, 4 * H], bf16)          # [128, 256]
        nc.scalar.dma_start(wq_sb[:, :], wqT[:, :])
        wp_sb = wpool.tile([H + 1, D], bf16)            # [65, 512]
        nc.scalar.dma_start(wp_sb[:, :], wpT[:, :])
        bkv_sb = wpool.tile([1, 2 * H], bf16)
        nc.scalar.dma_start(bkv_sb[:, :], bkv[:, :])
        bq_sb = wpool.tile([H, 1], f32)
        nc.scalar.dma_start(bq_sb[:, :], bqv[:, :])
        ones_sb = wpool.tile([1, 128], bf16)
        nc.scalar.dma_start(ones_sb[:, :], ones[:, :])
        antiI_sb = wpool.tile([128, 128], f32, name="antiI")
        nc.scalar.dma_start(antiI_sb[:, :], antiI[:, :])
        col64 = wpool.tile([128, 1], bf16, name="col64")
        nc.gpsimd.memset(col64[:, :], SCALE)
        one1 = wpool.tile([1, 1], f32, name="one1")
        nc.gpsimd.memset(one1[:, :], 1.0)
        warm = wpool.tile([128, 512], bf16, name="warm")
        nc.gpsimd.memset(warm[:, :], 1.0)

        # --- input streams: x halves in consumption order on sync; r on
        # gpsimd, held back behind the first 4MB of x ---
        from concourse.tile import add_dep_helper
        xh = [[None] * NH for _ in range(BPC)]
        rr = [None] * NCP
        x_dmas = []

        for j in range(NH):
            for b in range(BPC):
                t_ = xpool.tile([128, 2048], bf16, name=f"x{b}h{j}",
                                tag="xg")
                x_dmas.append(nc.sync.dma_start(t_[:, :], xp[b, j]))
                xh[b][j] = t_
        for cp in range(NCP):
            t_ = rpool.tile([128, 2 * TPC], f8, name=f"r{cp}", tag="rr")
            rd = nc.gpsimd.dma_start(t_[:, :], rp[cp])
            if cp == 0:
                add_dep_helper(rd.ins, x_dmas[5].ins,
                               reason="delay r stream behind first 3MB of x")
            rr[cp] = t_

        z_sb = [zpool.tile([128, 32 * 128], bf16, name=f"z{b}", tag="z")
                for b in range(BPC)]
        zq_sb = [zqpool.tile([128, 32 * 128], f8, name=f"zq{b}", tag="zq")
                 for b in range(BPC)]
        sq_sb = [sqpool.tile([H, TPC], f32, name=f"sq{b}", tag="sq")
                 for b in range(BPC)]
        # one full PSUM bank per batch: start_tensor_calc zeroes a 2KB
        # region, so interleaved accumulation groups must not share a bank
        cs_ps = [pcs.tile([128, 512], f32, name=f"cs{b}", tag="cs")
                 for b in range(BPC)]

        # --- phase 1 at (batch, 512-s-block) granularity: Z = [eKV | eK],
        # colsum via tiny z-as-weights matmuls, interleaved fp8 copy ---
        def hb(b, j):
            pkv = pps.tile([128, 512], f32, name="pkv", tag="ps")
            for si in range(4):
                for d in range(4):
                    nc.tensor.matmul(
                        pkv[:, si * 128:(si + 1) * 128],
                        lhsT=xh[b][j][:, d * 512 + si * 128:
                                      d * 512 + si * 128 + 128],
                        rhs=wkv_sb[:, d * 128:(d + 1) * 128],
                        start=(d == 0),
                        stop=(not use_bias and d == 3))
                if use_bias:
                    nc.tensor.matmul(
                        pkv[:, si * 128:(si + 1) * 128],
                        lhsT=ones_sb[:, :], rhs=bkv_sb[:, :],
                        start=False, stop=True)
            c0 = j * 512
            pk3 = pkv[:, :].rearrange("p (c k) -> p c k", c=4)
            zg3 = z_sb[b][:, c0:c0 + 512].rearrange("p (c k) -> p c k", c=4)
            nc.scalar.activation(
                zg3[:, :, H:2 * H], pk3[:, :, 0:H], Exp)
            nc.vector.tensor_tensor(
                zg3[:, :, 0:H], pk3[:, :, H:2 * H],
                zg3[:, :, H:2 * H], mult)
            # fp8 copy in DoubleRowSwInterleave weights layout:
            # zq[p, c0 + q*256 + 2*k + i] = z[p, c0 + (2q+i)*128 + k]
            zsrc = z_sb[b][:, c0:c0 + 512].rearrange(
                "p (q i k) -> p q i k", q=2, i=2)
            zdst = zq_sb[b][:, c0:c0 + 512].rearrange(
                "p (q k i) -> p q k i", q=2, k=128)
            for i in range(2):
                nc.vector.tensor_copy(zdst[:, :, :, i], zsrc[:, :, i, :])
            for si in range(4):
                sc = j * 4 + si
                nc.tensor.matmul(
                    cs_ps[b][:, 0:1],
                    lhsT=z_sb[b][:, sc * 128:(sc + 1) * 128],
                    rhs=col64[:, :],
                    start=(sc == 0), stop=(sc == 31),
                    skip_group_check=True)

        def q_block(b, j):
            pq = pps.tile([128, 512], f32, name="pq", tag="ps")
            for d in range(4):
                nc.tensor.matmul(
                    pq[0:H, :],
                    lhsT=wq_sb[:, d * H:(d + 1) * H],
                    rhs=xh[b][j][:, d * 512:(d + 1) * 512],
                    start=(d == 0), stop=(d == 3))
            nc.scalar.activation(
                sq_sb[b][:, j * 512:(j + 1) * 512], pq[0:H, :],
                Sigmoid, bias=bq_sb[:, :])

        # --- phase 2: nd[p][b] += zq_cp^T @ r_cp over all 16 chunk-pairs
        # (fp8e4m3 DoubleRowSwInterleave: 256-deep contraction, 2x rate) ---
        nd = [[None] * BPC for _ in range(NP)]

        def ph2_cp(cp, passes):
            rr3 = rr[cp][:, :].rearrange("p (two t) -> p two t", two=2)
            for b in range(BPC):
                zq2 = zq_sb[b][:, cp * 256:(cp + 1) * 256]
                for p in passes:
                    nc.tensor.matmul(
                        nd[p][b][:, :],
                        lhsT=zq2[:, :],
                        rhs=rr3[:, :, p * TP:(p + 1) * TP],
                        start=(cp == 0), stop=(cp == NCP - 1),
                        perf_mode=DR)

        # --- phase 3 --- (SwInterleave reverses nd partitions: partition m
        # holds natural column 127-m, so [0:H]=den-rev, [H:2H]=num-rev and
        # csv is reversed to match via the antiI matmul)
        csv = cpool.tile([128, BPC], f32, name="csv", tag="csv")
        csr = cpool.tile([128, BPC], f32, name="csr", tag="csr")
        tmps = {}

        add = mybir.AluOpType.add

        def ph3_reads(p, b):
            ncp = tpool.tile([H, TP], f32, name="ncp", tag="ncp")
            nc.vector.tensor_scalar(ncp[:, :], nd[p][b][H:2 * H, :],
                                    csv[H:2 * H, b:b + 1], None, add)
            dcp = tpool.tile([H, TP], f32, name="dcp", tag="dcp")
            nc.scalar.activation(dcp[:, :], nd[p][b][0:H, :], Identity,
                                 bias=csv[0:H, b:b + 1])
            rec = tpool.tile([H, TP], f32, name="rec", tag="rec")
            nc.vector.reciprocal_approx_fast(rec[:, :], dcp[:, :])
            tmp = tpool.tile([H, TP], f32, name="tmp", tag="tmp", bufs=8)
            nc.vector.tensor_tensor(tmp[:, :], ncp[:, :], rec[:, :], mult)
            tmps[(p, b)] = tmp

        def ph3_tail(p, b):
            yt = ytpool.tile([H + 1, TP], bf16, name="yt", tag="yt")
            nc.gpsimd.memset(yt[H:H + 1, :], 1.0)
            nc.gpsimd.tensor_tensor(
                yt[0:H, :], tmps[(p, b)][:, :],
                sq_sb[b][:, p * TP:(p + 1) * TP], mult)
            for c in range(4):
                po = pps.tile([128, 512], f32, name="po", tag="ps")
                nc.tensor.matmul(po[:, :], lhsT=yt[:, c * 128:(c + 1) * 128],
                                 rhs=wp_sb[:, :], start=True, stop=True)
                osb = opool.tile([128, 512], bf16, name="osb", tag="osb")
                eng_c = (nc.scalar.copy, nc.vector.tensor_copy,
                         nc.scalar.copy, nc.vector.tensor_copy)[c]
                eng_c(osb[:, :], po[:, :])
                t0 = p * TP + c * 128
                eng = (nc.gpsimd, nc.sync)[c % 2]
                eng.dma_start(out[b, t0:t0 + 128, :], osb[:, :])

        # --- emission schedule (PE order = program order per engine) ---
        # warmup runway: ramp the PE clock while the DMA engines start up
        warm_ps = pps.tile([128, 512], f32, name="warm_ps", tag="ps")
        for w in range(16):
            nc.tensor.matmul(warm_ps[:, :], lhsT=warm[:, 0:128],
                             rhs=warm[:, :], start=True, stop=True,
                             skip_group_check=True)
        # phase 1 paced to the x half-tile arrival order, hedged with early
        # phase-2 chunk-pairs once their z chunks exist (if x lags, PE does
        # ph2; if r lags, PE does ph1)
        for j in range(6):
            hb(0, j)
            hb(1, j)
        for p in (0, 1):
            for b in range(BPC):
                nd[p][b] = pnd.tile([128, TP], f32, name=f"nd{p}{b}",
                                    tag="nd")
        ph2_cp(0, (0, 1))
        ph2_cp(1, (0, 1))
        hb(0, 6)
        hb(1, 6)
        ph2_cp(2, (0, 1))
        ph2_cp(3, (0, 1))
        hb(0, 7)
        hb(1, 7)
        # csv: copy colsums out of PSUM, reverse via antiI matmul
        for b in range(BPC):
            nc.scalar.copy(csr[:, b:b + 1], cs_ps[b][:, 0:1])
            pcs_r = pps.tile([128, 512], f32, name="pcsr", tag="ps")
            nc.tensor.matmul(pcs_r[:, 0:1], lhsT=antiI_sb[:, :],
                             rhs=csr[:, b:b + 1], start=True, stop=True)
            nc.scalar.copy(csv[:, b:b + 1], pcs_r[:, 0:1])
        # rest of the (0,1) stream with Q blocks as r-wait filler
        for cp in range(4, NCP):
            ph2_cp(cp, (0, 1))
            if cp < 12:
                q_block((cp - 4) % 2, (cp - 4) // 2)
        ph3_reads(0, 0)
        ph3_reads(0, 1)
        ph3_reads(1, 0)
        ph3_reads(1, 1)
        # passes (2,3) together from resident r tiles: nd2 rotates onto
        # nd0's banks (WAR on reads(0)), nd3 into the freed colsum banks
        for b in range(BPC):
            nd[2][b] = pnd.tile([128, TP], f32, name=f"nd2{b}", tag="nd")
        for b in range(BPC):
            nd[3][b] = pcs.tile([128, 512], f32, name=f"nd3{b}", tag="cs")
        for cp in range(NCP):
            ph2_cp(cp, (2, 3))
        ph3_reads(2, 0)
        ph3_reads(2, 1)
        ph3_reads(3, 0)
        ph3_reads(3, 1)
        ph3_tail(0, 0)
        ph3_tail(0, 1)
        ph3_tail(1, 0)
        ph3_tail(1, 1)
        ph3_tail(2, 0)
        ph3_tail(2, 1)
        ph3_tail(3, 0)
        ph3_tail(3, 1)

    nc.compile()
    from concourse.bass_interp import get_hw_module
    nc.m = get_hw_module(nc.m)
    return nc


def _get_module(use_bias):
    key = ("nc", use_bias)
    if key not in _NC_CACHE:
        _NC_CACHE[key] = _build_module(use_bias)
    return _NC_CACHE[key]


def kernel(x, wq, bq, wk, bk, wv, bv, wp, bp, wbias):
    from concourse.bass_utils import run_bass_kernel_spmd

    bf16 = ml_dtypes.bfloat16
    f8 = ml_dtypes.float8_e4m3
    x = np.asarray(x, np.float32)
    wbias = np.asarray(wbias, np.float32)
    wq, wk, wv, wp = (np.asarray(a, np.float32) for a in (wq, wk, wv, wp))
    bq, bk, bv, bp = (np.asarray(a, np.float32) for a in (bq, bk, bv, bp))

    xT_full = np.ascontiguousarray(x.transpose(0, 2, 1)).astype(bf16)
    rq_full = (SCALE * (np.exp(wbias) - 1.0)).T     # [s, t] f32

    # pack the d-chunked SBUF layouts host-side
    wkv_h = np.ascontiguousarray(
        np.concatenate([wk.T, wv.T], axis=1).astype(bf16)
        .reshape(4, 128, 2 * H).transpose(1, 0, 2).reshape(128, 4 * 2 * H))
    # h-reversed Q/projection weights: the SwInterleave phase-2 matmul emits
    # nd with reversed partitions, so sq/tmp/yt all live in reversed-h order
    wqT_h = np.ascontiguousarray(
        wq.T[:, ::-1].astype(bf16).reshape(4, 128, H).transpose(1, 0, 2)
        .reshape(128, 4 * H))
    wpT_h = np.concatenate(
        [wp.T[::-1], np.asarray(bp, np.float32)[None, :]],
        axis=0).astype(bf16)
    bkv_h = np.concatenate([bk, bv])[None, :].astype(bf16)         # [1, 2H]
    bq_h = np.asarray(bq, np.float32)[::-1, None].copy()           # [H, 1]
    ones_h = np.ones((1, 128), dtype=bf16)
    antiI_h = np.eye(128, dtype=np.float32)[:, ::-1].copy()        # [128,128]
    use_bias = bool(np.any(bk) or np.any(bv))

    # Per t-group: s-permuted inputs (own t-slice rows first) so the SPMD
    # graph reads Q's x columns at [0:TPC] on every core.
    perm = {}
    for tj in range(TG):
        perm[tj] = np.concatenate([
            np.arange(tj * TPC, (tj + 1) * TPC),
            np.arange(0, tj * TPC),
            np.arange((tj + 1) * TPC, T)])

    # r chunk-pair layout for DoubleRow*: rp[cp][p, i*TPC + t] =
    # r[s = 256*cp + 128*i + p, t]
    rp_tj = {}
    for tj in range(TG):
        rq = rq_full[perm[tj]][:, tj * TPC:(tj + 1) * TPC].astype(f8)
        rp_tj[tj] = np.ascontiguousarray(
            rq.reshape(NCP, 2, 128, TPC).transpose(0, 2, 1, 3)
            .reshape(NCP, 128, 2 * TPC))

    # x half-tiles: xp[b, j, p, d*512 + s'] = xT[b, d*128+p, j*512+s']
    xp_c = {}
    for bi in range(BG):
        for tj in range(TG):
            xt = xT_full[bi * BPC:(bi + 1) * BPC][:, :, perm[tj]]
            xp_c[(bi, tj)] = np.ascontiguousarray(
                xt.reshape(BPC, 4, 128, NH, 512).transpose(0, 3, 2, 1, 4)
                .reshape(BPC, NH, 128, 2048))

    in_maps = []
    for c in range(N_CORES):
        bi, tj = c // TG, c % TG
        in_maps.append({
            "xp": xp_c[(bi, tj)],
            "rp": rp_tj[tj],
            "wkv": wkv_h, "wqT": wqT_h, "wpT": wpT_h,
            "bkv": bkv_h, "bqv": bq_h, "ones": ones_h, "antiI": antiI_h,
        })

    nc = _get_module(use_bias)
    res = run_bass_kernel_spmd(nc, in_maps, core_ids=list(range(N_CORES)))

    full = np.empty((B, T, D), dtype=np.float32)
    for c in range(N_CORES):
        bi, tj = c // TG, c % TG
        full[bi * BPC:(bi + 1) * BPC, tj * TPC:(tj + 1) * TPC, :] = \
            res.results[c]["out"].astype(np.float32)
    return full


# revision 26
# speedup vs baseline: 1.0660x; 1.0077x over previous
"""AFT-Full distributed Trainium2 kernel (v5).

Reference computation (B=8, T=4096, D=512, H=64):
    Q = x @ wq.T ; K = x @ wk.T ; V = x @ wv.T                     [B,T,H]
    ew  = exp(wbias)                                               [T,T]
    num = ew @ (exp(K)*V) ; den = ew @ exp(K)                      [B,T,H]
    out = (sigmoid(Q) * num/den) @ wp.T + bp                       [B,T,D]

Sharding: 4 batch-groups x 2 t-groups (core c: batches {2*(c//2),
2*(c//2)+1}, t-slice c%2).  No collectives.

v5 vs the 114us v4 (SwInterleave):
- x ships in 512KB half-tiles consumed at (batch, 512-s-block)
  granularity, so phase 1 starts ~1.4us after launch and never waits a
  full 1MB tile.
- all of phase 1 + Q + colsum run first (PE-paced inside the x-DMA
  window) while r streams behind x; phase 2 then runs as one stream
  over chunk-pairs with THREE t-passes live (pass 2 reuses the freed
  colsum PSUM banks), matching PE consumption to r arrival with
  SBUF-buffered slack.  Pass 3 re-reads resident r tiles and is
  spliced with the phase-3 epilogues of passes 0-2.
- phase-2 matmuls are fp8e4m3 DoubleRowSwInterleave (256-deep
  contraction at 2x rate); the fp8 z copy is written directly in the
  interleaved weights layout by strided DVE casts.  The plain
  DoubleRow perf mode NaNs on HW through this toolchain; SwInterleave
  (software-interleaved weights) works.  The interleave's inherent
  column reversal makes nd partitions come out h-reversed: absorbed by
  h-reversed wq/bq/wp.T host packing and an on-chip csv reversal via
  one f32 matmul against a host-shipped anti-identity.
- exact colsum (the bulk of num/den) still reads bf16 z, so fp8 error
  only touches the small ew-1 correction term.
"""

import sys

for _p in ("/opt/trn_rl_repo", "/opt/pypackages"):
    if _p not in sys.path:
        sys.path.append(_p)

import numpy as np
import ml_dtypes

B, T, D, H = 8, 4096, 512, 64
BG, TG = 4, 2            # batch groups x t groups = 8 cores
BPC = B // BG            # batches per core
TPC = T // TG            # t rows per core
NH = 8                   # x s-halves of 512
NCP = 16                 # r chunk-pairs of 256 s rows
NP = 4                   # phase-2/3 t-passes of 512
TP = TPC // NP
SCALE = 64.0
N_CORES = 8

_NC_CACHE = {}


def _build_module(use_bias):
    import concourse.bass as bass
    import concourse.mybir as mybir
    import concourse.tile as tile
    from concourse import bacc
    from contextlib import ExitStack

    bf16 = mybir.dt.bfloat16
    f32 = mybir.dt.float32
    f8 = mybir.dt.float8e4
    DR = mybir.MatmulPerfMode.DoubleRowSwInterleave
    Exp = mybir.ActivationFunctionType.Exp
    Sigmoid = mybir.ActivationFunctionType.Sigmoid
    Identity = mybir.ActivationFunctionType.Identity
    mult = mybir.AluOpType.mult

    nc = bacc.Bacc("TRN2", target_bir_lowering=False, debug=False,
                   num_devices=N_CORES)

    xp = nc.dram_tensor("xp", [BPC, NH, 128, 2048], bf16,
                        kind="ExternalInput").ap()
    rp = nc.dram_tensor("rp", [NCP, 128, 2 * TPC], f8,
                        kind="ExternalInput").ap()
    wkv = nc.dram_tensor("wkv", [128, 4 * 2 * H], bf16,
                         kind="ExternalInput").ap()
    wqT = nc.dram_tensor("wqT", [128, 4 * H], bf16,
                         kind="ExternalInput").ap()
    wpT = nc.dram_tensor("wpT", [H + 1, D], bf16, kind="ExternalInput").ap()
    bkv = nc.dram_tensor("bkv", [1, 2 * H], bf16, kind="ExternalInput").ap()
    bqv = nc.dram_tensor("bqv", [H, 1], f32, kind="ExternalInput").ap()
    ones = nc.dram_tensor("ones", [1, 128], bf16, kind="ExternalInput").ap()
    antiI = nc.dram_tensor("antiI", [128, 128], f32,
                           kind="ExternalInput").ap()
    out = nc.dram_tensor("out", [BPC, TPC, D], bf16,
                         kind="ExternalOutput").ap()

    with tile.TileContext(nc) as tc, ExitStack() as ctx:
        wpool = ctx.enter_context(tc.tile_pool(name="wts", bufs=1))
        xpool = ctx.enter_context(tc.tile_pool(name="xg", bufs=2 * NH))
        rpool = ctx.enter_context(tc.tile_pool(name="rr", bufs=NCP))
        zpool = ctx.enter_context(tc.tile_pool(name="z", bufs=BPC))
        zqpool = ctx.enter_context(tc.tile_pool(name="zq", bufs=BPC))
        sqpool = ctx.enter_context(tc.tile_pool(name="sq", bufs=BPC))
        cpool = ctx.enter_context(tc.tile_pool(name="csv", bufs=1))
        ytpool = ctx.enter_context(tc.tile_pool(name="yt", bufs=3))
        tpool = ctx.enter_context(tc.tile_pool(name="tmp", bufs=2))
        opool = ctx.enter_context(tc.tile_pool(name="osb", bufs=4))
        pps = ctx.enter_context(tc.tile_pool(name="pps", bufs=2,
                                             space="PSUM"))
        pnd = ctx.enter_context(tc.tile_pool(name="pnd", bufs=4,
                                             space="PSUM"))
        pcs = ctx.enter_context(tc.tile_pool(name="pcs", bufs=2,
                                             space="PSUM"))

        # --- resident weights / constants on the scalar queue ---
        wkv_sb = wpool.tile([128, 4 * 2 * H], bf16)     # [128, 512]
        nc.scalar.dma_start(wkv_sb[:, :], wkv[:, :])
        wq_sb = wpool.tile([128, 4 * H], bf16)          # [128, 256]
        nc.scalar.dma_start(wq_sb[:, :], wqT[:, :])
        wp_sb = wpool.tile([H + 1, D], bf16)            # [65, 512]
        nc.scalar.dma_start(wp_sb[:, :], wpT[:, :])
        bkv_sb = wpool.tile([1, 2 * H], bf16)
        nc.scalar.dma_start(bkv_sb[:, :], bkv[:, :])
        bq_sb = wpool.tile([H, 1], f32)
        nc.scalar.dma_start(bq_sb[:, :], bqv[:, :])
        ones_sb = wpool.tile([1, 128], bf16)
        nc.scalar.dma_start(ones_sb[:, :], ones[:, :])
        antiI_sb = wpool.tile([128, 128], f32, name="antiI")
        nc.scalar.dma_start(antiI_sb[:, :], antiI[:, :])
        col64 = wpool.tile([128, 1], bf16, name="col64")
        nc.gpsimd.memset(col64[:, :], SCALE)
        one1 = wpool.tile([1, 1], f32, name="one1")
        nc.gpsimd.memset(one1[:, :], 1.0)
        warm = wpool.tile([128, 512], bf16, name="warm")
        nc.gpsimd.memset(warm[:, :], 1.0)

        # --- input streams: x halves in consumption order on sync; r on
        # gpsimd, held back behind the first 4MB of x ---
        from concourse.tile import add_dep_helper
        xh = [[None] * NH for _ in range(BPC)]
        rr = [None] * NCP
        x_dmas = []

        for j in range(NH):
            for b in range(BPC):
                t_ = xpool.tile([128, 2048], bf16, name=f"x{b}h{j}",
                                tag="xg")
                x_dmas.append(nc.sync.dma_start(t_[:, :], xp[b, j]))
                xh[b][j] = t_
        for cp in range(NCP):
            t_ = rpool.tile([128, 2 * TPC], f8, name=f"r{cp}", tag="rr")
            rd = nc.gpsimd.dma_start(t_[:, :], rp[cp])
            if cp == 0:
                add_dep_helper(rd.ins, x_dmas[5].ins,
                               reason="delay r stream behind first 3MB of x")
            rr[cp] = t_

        z_sb = [zpool.tile([128, 32 * 128], bf16, name=f"z{b}", tag="z")
                for b in range(BPC)]
        zq_sb = [zqpool.tile([128, 32 * 128], f8, name=f"zq{b}", tag="zq")
                 for b in range(BPC)]
        sq_sb = [sqpool.tile([H, TPC], f32, name=f"sq{b}", tag="sq")
                 for b in range(BPC)]
        # one full PSUM bank per batch: start_tensor_calc zeroes a 2KB
        # region, so interleaved accumulation groups must not share a bank
        cs_ps = [pcs.tile([128, 512], f32, name=f"cs{b}", tag="cs")
                 for b in range(BPC)]

        # --- phase 1 at (batch, 512-s-block) granularity: Z = [eKV | eK],
        # colsum via tiny z-as-weights matmuls, interleaved fp8 copy ---
        def hb(b, j):
            pkv = pps.tile([128, 512], f32, name="pkv", tag="ps")
            for si in range(4):
                for d in range(4):
                    nc.tensor.matmul(
                        pkv[:, si * 128:(si + 1) * 128],
                        lhsT=xh[b][j][:, d * 512 + si * 128:
                                      d * 512 + si * 128 + 128],
                        rhs=wkv_sb[:, d * 128:(d + 1) * 128],
                        start=(d == 0),
                        stop=(not use_bias and d == 3))
                if use_bias:
                    nc.tensor.matmul(
                        pkv[:, si * 128:(si + 1) * 128],
                        lhsT=ones_sb[:, :], rhs=bkv_sb[:, :],
                        start=False, stop=True)
            c0 = j * 512
            pk3 = pkv[:, :].rearrange("p (c k) -> p c k", c=4)
            zg3 = z_sb[b][:, c0:c0 + 512].rearrange("p (c k) -> p c k", c=4)
            nc.scalar.activation(
                zg3[:, :, H:2 * H], pk3[:, :, 0:H], Exp)
            nc.vector.tensor_tensor(
                zg3[:, :, 0:H], pk3[:, :, H:2 * H],
                zg3[:, :, H:2 * H], mult)
            # fp8 copy in DoubleRowSwInterleave weights layout:
            # zq[p, c0 + q*256 + 2*k + i] = z[p, c0 + (2q+i)*128 + k]
            zsrc = z_sb[b][:, c0:c0 + 512].rearrange(
                "p (q i k) -> p q i k", q=2, i=2)
            zdst = zq_sb[b][:, c0:c0 + 512].rearrange(
                "p (q k i) -> p q k i", q=2, k=128)
            for i in range(2):
                nc.vector.tensor_copy(zdst[:, :, :, i], zsrc[:, :, i, :])
            for si in range(4):
                sc = j * 4 + si
                nc.tensor.matmul(
                    cs_ps[b][:, 0:1],
                    lhsT=z_sb[b][:, sc * 128:(sc + 1) * 128],
                    rhs=col64[:, :],
                    start=(sc == 0), stop=(sc == 31),
                    skip_group_check=True)

        def q_block(b, j):
            pq = pps.tile([128, 512], f32, name="pq", tag="ps")
            for d in range(4):
                nc.tensor.matmul(
                    pq[0:H, :],
                    lhsT=wq_sb[:, d * H:(d + 1) * H],
                    rhs=xh[b][j][:, d * 512:(d + 1) * 512],
                    start=(d == 0), stop=(d == 3))
            nc.scalar.activation(
                sq_sb[b][:, j * 512:(j + 1) * 512], pq[0:H, :],
                Sigmoid, bias=bq_sb[:, :])

        # --- phase 2: nd[p][b] += zq_cp^T @ r_cp over all 16 chunk-pairs
        # (fp8e4m3 DoubleRowSwInterleave: 256-deep contraction, 2x rate) ---
        nd = [[None] * BPC for _ in range(NP)]

        def ph2_cp(cp, passes):
            rr3 = rr[cp][:, :].rearrange("p (two t) -> p two t", two=2)
            for b in range(BPC):
                zq2 = zq_sb[b][:, cp * 256:(cp + 1) * 256]
                for p in passes:
                    nc.tensor.matmul(
                        nd[p][b][:, :],
                        lhsT=zq2[:, :],
                        rhs=rr3[:, :, p * TP:(p + 1) * TP],
                        start=(cp == 0), stop=(cp == NCP - 1),
                        perf_mode=DR)

        # --- phase 3 --- (SwInterleave reverses nd partitions: partition m
        # holds natural column 127-m, so [0:H]=den-rev, [H:2H]=num-rev and
        # csv is reversed to match via the antiI matmul)
        csv = cpool.tile([128, BPC], f32, name="csv", tag="csv")
        csr = cpool.tile([128, BPC], f32, name="csr", tag="csr")
        tmps = {}

        add = mybir.AluOpType.add

        def ph3_reads(p, b):
            ncp = tpool.tile([H, TP], f32, name="ncp", tag="ncp")
            nc.vector.tensor_scalar(ncp[:, :], nd[p][b][H:2 * H, :],
                                    csv[H:2 * H, b:b + 1], None, add)
            dcp = tpool.tile([H, TP], f32, name="dcp", tag="dcp")
            nc.scalar.activation(dcp[:, :], nd[p][b][0:H, :], Identity,
                                 bias=csv[0:H, b:b + 1])
            rec = tpool.tile([H, TP], f32, name="rec", tag="rec")
            nc.vector.reciprocal_approx_fast(rec[:, :], dcp[:, :])
            tmp = tpool.tile([H, TP], f32, name="tmp", tag="tmp", bufs=8)
            nc.vector.tensor_tensor(tmp[:, :], ncp[:, :], rec[:, :], mult)
            tmps[(p, b)] = tmp

        def ph3_tail(p, b):
            yt = ytpool.tile([H + 1, TP], bf16, name="yt", tag="yt")
            nc.gpsimd.memset(yt[H:H + 1, :], 1.0)
            nc.gpsimd.tensor_tensor(
                yt[0:H, :], tmps[(p, b)][:, :],
                sq_sb[b][:, p * TP:(p + 1) * TP], mult)
            for c in range(4):
                po = pps.tile([128, 512], f32, name="po", tag="ps")
                nc.tensor.matmul(po[:, :], lhsT=yt[:, c * 128:(c + 1) * 128],
                                 rhs=wp_sb[:, :], start=True, stop=True)
                osb = opool.tile([128, 512], bf16, name="osb", tag="osb")
                eng_c = (nc.scalar.copy, nc.vector.tensor_copy,
                         nc.scalar.copy, nc.vector.tensor_copy)[c]
                eng_c(osb[:, :], po[:, :])
                t0 = p * TP + c * 128
                eng = (nc.gpsimd, nc.sync)[c % 2]
                eng.dma_start(out[b, t0:t0 + 128, :], osb[:, :])

        # --- emission schedule (PE order = program order per engine) ---
        # warmup runway: ramp the PE clock while the DMA engines start up
        warm_ps = pps.tile([128, 512], f32, name="warm_ps", tag="ps")
        for w in range(4):
            nc.tensor.matmul(warm_ps[:, :], lhsT=warm[:, 0:128],
                             rhs=warm[:, :], start=True, stop=True,
                             skip_group_check=True)
        # phase 1 paced to the x half-tile arrival order, hedged with early
        # phase-2 chunk-pairs once their z chunks exist (if x lags, PE does
        # ph2; if r lags, PE does ph1)
        for j in range(6):
            hb(0, j)
            hb(1, j)
        for p in (0, 1):
            for b in range(BPC):
                nd[p][b] = pnd.tile([128, TP], f32, name=f"nd{p}{b}",
                                    tag="nd")
        ph2_cp(0, (0, 1))
        ph2_cp(1, (0, 1))
        hb(0, 6)
        hb(1, 6)
        ph2_cp(2, (0, 1))
        ph2_cp(3, (0, 1))
        hb(0, 7)
        hb(1, 7)
        # csv: copy colsums out of PSUM, reverse via antiI matmul
        for b in range(BPC):
            nc.scalar.copy(csr[:, b:b + 1], cs_ps[b][:, 0:1])
            pcs_r = pps.tile([128, 512], f32, name="pcsr", tag="ps")
            nc.tensor.matmul(pcs_r[:, 0:1], lhsT=antiI_sb[:, :],
                             rhs=csr[:, b:b + 1], start=True, stop=True)
            nc.scalar.copy(csv[:, b:b + 1], pcs_r[:, 0:1])
        # rest of the (0,1) stream with Q blocks as r-wait filler
        for cp in range(4, NCP):
            ph2_cp(cp, (0, 1))
            if cp < 12:
                q_block((cp - 4) % 2, (cp - 4) // 2)
        ph3_reads(0, 0)
        ph3_reads(0, 1)
        ph3_reads(1, 0)
        ph3_reads(1, 1)
        # passes (2,3) together from resident r tiles: nd2 rotates onto
        # nd0's banks (WAR on reads(0)), nd3 into the freed colsum banks
        for b in range(BPC):
            nd[2][b] = pnd.tile([128, TP], f32, name=f"nd2{b}", tag="nd")
        for b in range(BPC):
            nd[3][b] = pcs.tile([128, 512], f32, name=f"nd3{b}", tag="cs")
        for cp in range(0, 6):
            ph2_cp(cp, (2, 3))
        ph3_tail(0, 0)
        ph3_tail(0, 1)
        for cp in range(6, 12):
            ph2_cp(cp, (2, 3))
        ph3_tail(1, 0)
        ph3_tail(1, 1)
        for cp in range(12, NCP):
            ph2_cp(cp, (2, 3))
        ph3_reads(2, 0)
        ph3_reads(2, 1)
        ph3_reads(3, 0)
        ph3_reads(3, 1)
        ph3_tail(2, 0)
        ph3_tail(2, 1)
        ph3_tail(3, 0)
        ph3_tail(3, 1)

    nc.compile()
    from concourse.bass_interp import get_hw_module
    nc.m = get_hw_module(nc.m)
    return nc


def _get_module(use_bias):
    key = ("nc", use_bias)
    if key not in _NC_CACHE:
        _NC_CACHE[key] = _build_module(use_bias)
    return _NC_CACHE[key]


def kernel(x, wq, bq, wk, bk, wv, bv, wp, bp, wbias):
    from concourse.bass_utils import run_bass_kernel_spmd

    bf16 = ml_dtypes.bfloat16
    f8 = ml_dtypes.float8_e4m3
    x = np.asarray(x, np.float32)
    wbias = np.asarray(wbias, np.float32)
    wq, wk, wv, wp = (np.asarray(a, np.float32) for a in (wq, wk, wv, wp))
    bq, bk, bv, bp = (np.asarray(a, np.float32) for a in (bq, bk, bv, bp))

    xT_full = np.ascontiguousarray(x.transpose(0, 2, 1)).astype(bf16)
    rq_full = (SCALE * (np.exp(wbias) - 1.0)).T     # [s, t] f32

    # pack the d-chunked SBUF layouts host-side
    wkv_h = np.ascontiguousarray(
        np.concatenate([wk.T, wv.T], axis=1).astype(bf16)
        .reshape(4, 128, 2 * H).transpose(1, 0, 2).reshape(128, 4 * 2 * H))
    # h-reversed Q/projection weights: the SwInterleave phase-2 matmul emits
    # nd with reversed partitions, so sq/tmp/yt all live in reversed-h order
    wqT_h = np.ascontiguousarray(
        wq.T[:, ::-1].astype(bf16).reshape(4, 128, H).transpose(1, 0, 2)
        .reshape(128, 4 * H))
    wpT_h = np.concatenate(
        [wp.T[::-1], np.asarray(bp, np.float32)[None, :]],
        axis=0).astype(bf16)
    bkv_h = np.concatenate([bk, bv])[None, :].astype(bf16)         # [1, 2H]
    bq_h = np.asarray(bq, np.float32)[::-1, None].copy()           # [H, 1]
    ones_h = np.ones((1, 128), dtype=bf16)
    antiI_h = np.eye(128, dtype=np.float32)[:, ::-1].copy()        # [128,128]
    use_bias = bool(np.any(bk) or np.any(bv))

    # Per t-group: s-permuted inputs (own t-slice rows first) so the SPMD
    # graph reads Q's x columns at [0:TPC] on every core.
    perm = {}
    for tj in range(TG):
        perm[tj] = np.concatenate([
            np.arange(tj * TPC, (tj + 1) * TPC),
            np.arange(0, tj * TPC),
            np.arange((tj + 1) * TPC, T)])

    # r chunk-pair layout for DoubleRow*: rp[cp][p, i*TPC + t] =
    # r[s = 256*cp + 128*i + p, t]
    rp_tj = {}
    for tj in range(TG):
        rq = rq_full[perm[tj]][:, tj * TPC:(tj + 1) * TPC].astype(f8)
        rp_tj[tj] = np.ascontiguousarray(
            rq.reshape(NCP, 2, 128, TPC).transpose(0, 2, 1, 3)
            .reshape(NCP, 128, 2 * TPC))

    # x half-tiles: xp[b, j, p, d*512 + s'] = xT[b, d*128+p, j*512+s']
    xp_c = {}
    for bi in range(BG):
        for tj in range(TG):
            xt = xT_full[bi * BPC:(bi + 1) * BPC][:, :, perm[tj]]
            xp_c[(bi, tj)] = np.ascontiguousarray(
                xt.reshape(BPC, 4, 128, NH, 512).transpose(0, 3, 2, 1, 4)
                .reshape(BPC, NH, 128, 2048))

    in_maps = []
    for c in range(N_CORES):
        bi, tj = c // TG, c % TG
        in_maps.append({
            "xp": xp_c[(bi, tj)],
            "rp": rp_tj[tj],
            "wkv": wkv_h, "wqT": wqT_h, "wpT": wpT_h,
            "bkv": bkv_h, "bqv": bq_h, "ones": ones_h, "antiI": antiI_h,
        })

    nc = _get_module(use_bias)
    res = run_bass_kernel_spmd(nc, in_maps, core_ids=list(range(N_CORES)))

    full = np.empty((B, T, D), dtype=np.float32)
    for c in range(N_CORES):
        bi, tj = c // TG, c % TG
        full[bi * BPC:(bi + 1) * BPC, tj * TPC:(tj + 1) * TPC, :] = \
            res.results[c]["out"].astype(np.float32)
    return full


# revision 27
# speedup vs baseline: 1.1328x; 1.0627x over previous
"""AFT-Full distributed Trainium2 kernel (v4: fp8e4m3 DoubleRowSwInterleave
phase 2 on the v2 interleaved schedule).

Reference computation (B=8, T=4096, D=512, H=64):
    Q = x @ wq.T ; K = x @ wk.T ; V = x @ wv.T                     [B,T,H]
    ew  = exp(wbias)                                               [T,T]
    num = ew @ (exp(K)*V) ; den = ew @ exp(K)                      [B,T,H]
    out = (sigmoid(Q) * num/den) @ wp.T + bp                       [B,T,D]

Sharding: 4 batch-groups x 2 t-groups (core c: batches {2*(c//2),
2*(c//2)+1}, t-slice c%2).  No collectives.

Key structure:
- ew = 1 + r decomposition: host ships r = SCALE*(exp(wbias)-1) as
  fp8e4m3; num/den = exact colsum(z) (bf16 z via PE matmuls against a
  SCALE column) + z^T @ r correction.  The correction is small
  (O(1/sqrt(T)) of the total), so fp8 error on BOTH operands of the
  phase-2 matmul is negligible; the bulk flows through the exact
  colsum.  SCALE cancels in num/den.
- phase 2 runs as fp8e4m3 DoubleRowSwInterleave matmuls: 256-deep
  contraction per instruction at 2x rate (157 TF/s).  Plain DoubleRow
  NaNs on HW through this toolchain; SwInterleave (weights
  software-interleaved: mem[2f+i] = column f of s-chunk-parity i,
  logical column m = 127-f) works.  The fp8 z copy is written directly
  in that layout by strided DVE casts.  The inherent column reversal
  makes nd partitions h-reversed: absorbed by h-reversed wq/bq/wp.T
  host packing and an on-chip csv reversal via one f32 matmul against
  a host-shipped anti-identity.
- schedule interleaves phase-1 x-group consumption, Q, colsum, and the
  phase-2 chunk-pair stream so either DMA stream (x or r) can feed the
  PE while the other lags; phase-3 epilogues are spliced between
  phase-2 bursts of the later passes.
"""

import sys

for _p in ("/opt/trn_rl_repo", "/opt/pypackages"):
    if _p not in sys.path:
        sys.path.append(_p)

import numpy as np
import ml_dtypes

B, T, D, H = 8, 4096, 512, 64
BG, TG = 4, 2            # batch groups x t groups = 8 cores
BPC = B // BG            # batches per core
TPC = T // TG            # t rows per core
NG = 4                   # x s-groups of 1024
NCP = 16                 # r chunk-pairs of 256 s rows
NP = 4                   # phase-2/3 t-passes of 512
TP = TPC // NP
SCALE = 64.0
N_CORES = 8

_NC_CACHE = {}


def _build_module(use_bias):
    import concourse.bass as bass
    import concourse.mybir as mybir
    import concourse.tile as tile
    from concourse import bacc
    from contextlib import ExitStack

    bf16 = mybir.dt.bfloat16
    f32 = mybir.dt.float32
    f8 = mybir.dt.float8e4
    DR = mybir.MatmulPerfMode.DoubleRowSwInterleave
    Exp = mybir.ActivationFunctionType.Exp
    Sigmoid = mybir.ActivationFunctionType.Sigmoid
    Identity = mybir.ActivationFunctionType.Identity
    mult = mybir.AluOpType.mult

    nc = bacc.Bacc("TRN2", target_bir_lowering=False, debug=False,
                   num_devices=N_CORES)

    xp = nc.dram_tensor("xp", [BPC, NG, 128, 4096], bf16,
                        kind="ExternalInput").ap()
    rp = nc.dram_tensor("rp", [NCP, 128, 2 * TPC], f8,
                        kind="ExternalInput").ap()
    wkv = nc.dram_tensor("wkv", [128, 4 * 2 * H], bf16,
                         kind="ExternalInput").ap()
    wqT = nc.dram_tensor("wqT", [128, 4 * H], bf16,
                         kind="ExternalInput").ap()
    wpT = nc.dram_tensor("wpT", [H + 1, D], bf16, kind="ExternalInput").ap()
    bkv = nc.dram_tensor("bkv", [1, 2 * H], bf16, kind="ExternalInput").ap()
    bqv = nc.dram_tensor("bqv", [H, 1], f32, kind="ExternalInput").ap()
    ones = nc.dram_tensor("ones", [1, 128], bf16, kind="ExternalInput").ap()
    antiI = nc.dram_tensor("antiI", [128, 128], f32,
                           kind="ExternalInput").ap()
    out = nc.dram_tensor("out", [BPC, TPC, D], bf16,
                         kind="ExternalOutput").ap()

    with tile.TileContext(nc) as tc, ExitStack() as ctx:
        wpool = ctx.enter_context(tc.tile_pool(name="wts", bufs=1))
        xpool = ctx.enter_context(tc.tile_pool(name="xg", bufs=2 * NG))
        rpool = ctx.enter_context(tc.tile_pool(name="rr", bufs=NCP))
        zpool = ctx.enter_context(tc.tile_pool(name="z", bufs=BPC))
        zqpool = ctx.enter_context(tc.tile_pool(name="zq", bufs=BPC))
        sqpool = ctx.enter_context(tc.tile_pool(name="sq", bufs=BPC))
        cpool = ctx.enter_context(tc.tile_pool(name="csv", bufs=1))
        ytpool = ctx.enter_context(tc.tile_pool(name="yt", bufs=3))
        tpool = ctx.enter_context(tc.tile_pool(name="tmp", bufs=3))
        opool = ctx.enter_context(tc.tile_pool(name="osb", bufs=4))
        pps = ctx.enter_context(tc.tile_pool(name="pps", bufs=2,
                                             space="PSUM"))
        pnd = ctx.enter_context(tc.tile_pool(name="pnd", bufs=4,
                                             space="PSUM"))
        pcs = ctx.enter_context(tc.tile_pool(name="pcs", bufs=2,
                                             space="PSUM"))

        # --- resident weights / constants: host-packed single DMAs on the
        # scalar queue, so the sync queue starts programming x immediately ---
        wkv_sb = wpool.tile([128, 4 * 2 * H], bf16)     # [128, 512]
        nc.scalar.dma_start(wkv_sb[:, :], wkv[:, :])
        wq_sb = wpool.tile([128, 4 * H], bf16)          # [128, 256]
        nc.scalar.dma_start(wq_sb[:, :], wqT[:, :])
        wp_sb = wpool.tile([H + 1, D], bf16)            # [65, 512]
        nc.scalar.dma_start(wp_sb[:, :], wpT[:, :])
        bkv_sb = wpool.tile([1, 2 * H], bf16)
        nc.scalar.dma_start(bkv_sb[:, :], bkv[:, :])
        bq_sb = wpool.tile([H, 1], f32)
        nc.scalar.dma_start(bq_sb[:, :], bqv[:, :])
        ones_sb = wpool.tile([1, 128], bf16)
        nc.scalar.dma_start(ones_sb[:, :], ones[:, :])
        antiI_sb = wpool.tile([128, 128], f32, name="antiI")
        nc.scalar.dma_start(antiI_sb[:, :], antiI[:, :])
        col64 = wpool.tile([128, 1], bf16, name="col64")
        nc.gpsimd.memset(col64[:, :], SCALE)
        one1 = wpool.tile([1, 1], f32, name="one1")
        nc.gpsimd.memset(one1[:, :], 1.0)

        # --- input streams: x in consumption order on sync; r on gpsimd,
        # held back behind the first two x groups so phase 1 gets full
        # bandwidth at the start ---
        from concourse.tile import add_dep_helper
        xg = [[None] * NG for _ in range(BPC)]
        rr = [None] * NCP
        x_dmas = []

        def x_dma(g):
            for b in range(BPC):
                t_ = xpool.tile([128, 4096], bf16, name=f"x{b}g{g}",
                                tag="xg")
                x_dmas.append(nc.sync.dma_start(t_[:, :], xp[b, g]))
                xg[b][g] = t_

        for g in range(NG):
            x_dma(g)
        for cp in range(NCP):
            t_ = rpool.tile([128, 2 * TPC], f8, name=f"r{cp}", tag="rr")
            rd = nc.gpsimd.dma_start(t_[:, :], rp[cp])
            if cp == 0:
                add_dep_helper(rd.ins, x_dmas[3].ins,
                               reason="delay r stream behind x groups 0-1")
            rr[cp] = t_

        z_sb = [zpool.tile([128, 32 * 128], bf16, name=f"z{b}", tag="z")
                for b in range(BPC)]
        zq_sb = [zqpool.tile([128, 32 * 128], f8, name=f"zq{b}", tag="zq")
                 for b in range(BPC)]
        sq_sb = [sqpool.tile([H, TPC], f32, name=f"sq{b}", tag="sq")
                 for b in range(BPC)]
        # one full PSUM bank per batch: start_tensor_calc zeroes a 2KB
        # region, so interleaved accumulation groups must not share a bank
        cs_ps = [pcs.tile([128, 512], f32, name=f"cs{b}", tag="cs")
                 for b in range(BPC)]

        # --- phase 1: Z = [eKV | eK] per 512-s block; colsum via tiny
        # z-as-weights matmuls accumulating [2H,1] per batch; fp8 copy of
        # each finished 512-block in the SwInterleave weights layout ---
        def ph1_group(g):
            for b in range(BPC):
                for sg in range(2):
                    pkv = pps.tile([128, 512], f32, name="pkv", tag="ps")
                    for si in range(4):
                        xoff = (sg * 4 + si) * 128
                        for d in range(4):
                            nc.tensor.matmul(
                                pkv[:, si * 128:(si + 1) * 128],
                                lhsT=xg[b][g][:, d * 1024 + xoff:
                                              d * 1024 + xoff + 128],
                                rhs=wkv_sb[:, d * 128:(d + 1) * 128],
                                start=(d == 0),
                                stop=(not use_bias and d == 3))
                        if use_bias:
                            nc.tensor.matmul(
                                pkv[:, si * 128:(si + 1) * 128],
                                lhsT=ones_sb[:, :], rhs=bkv_sb[:, :],
                                start=False, stop=True)
                    c0 = (g * 8 + sg * 4) * 128
                    pk3 = pkv[:, :].rearrange("p (c k) -> p c k", c=4)
                    zg3 = z_sb[b][:, c0:c0 + 512].rearrange(
                        "p (c k) -> p c k", c=4)
                    nc.scalar.activation(
                        zg3[:, :, H:2 * H], pk3[:, :, 0:H], Exp)
                    nc.vector.tensor_tensor(
                        zg3[:, :, 0:H], pk3[:, :, H:2 * H],
                        zg3[:, :, H:2 * H], mult)
                    # fp8 copy in DoubleRowSwInterleave weights layout:
                    # zq[p, c0 + q*256 + 2*k + i] = z[p, c0 + (2q+i)*128 + k]
                    zsrc = z_sb[b][:, c0:c0 + 512].rearrange(
                        "p (q i k) -> p q i k", q=2, i=2)
                    zdst = zq_sb[b][:, c0:c0 + 512].rearrange(
                        "p (q k i) -> p q k i", q=2, k=128)
                    for i in range(2):
                        nc.vector.tensor_copy(
                            zdst[:, :, :, i], zsrc[:, :, i, :])
                for si in range(8):
                    sc = g * 8 + si
                    nc.tensor.matmul(
                        cs_ps[b][:, 0:1],
                        lhsT=z_sb[b][:, sc * 128:(sc + 1) * 128],
                        rhs=col64[:, :],
                        start=(sc == 0), stop=(sc == 31),
                        skip_group_check=True)

        def q_block(b):
            for qb in range(4):
                pq = pps.tile([128, 512], f32, name="pq", tag="ps")
                for d in range(4):
                    nc.tensor.matmul(
                        pq[0:H, :],
                        lhsT=wq_sb[:, d * H:(d + 1) * H],
                        rhs=xg[b][qb // 2][:, d * 1024 + (qb % 2) * 512:
                                           d * 1024 + (qb % 2) * 512 + 512],
                        start=(d == 0), stop=(d == 3))
                nc.scalar.activation(
                    sq_sb[b][:, qb * 512:(qb + 1) * 512], pq[0:H, :],
                    Sigmoid, bias=bq_sb[:, :])

        # --- phase 2: nd[p][b] += zq_cp^T @ r_cp over all 16 chunk-pairs
        # (fp8e4m3 DoubleRowSwInterleave: 256-deep contraction, 2x rate) ---
        nd = [[None] * BPC for _ in range(NP)]

        def nd_alloc(passes):
            for p in passes:
                for b in range(BPC):
                    nd[p][b] = pnd.tile([128, TP], f32, name=f"nd{p}{b}",
                                        tag="nd")

        def ph2_cp(cp, passes):
            rr3 = rr[cp][:, :].rearrange("p (two t) -> p two t", two=2)
            for b in range(BPC):
                zq2 = zq_sb[b][:, cp * 256:(cp + 1) * 256]
                for p in passes:
                    nc.tensor.matmul(
                        nd[p][b][:, :],
                        lhsT=zq2[:, :],
                        rhs=rr3[:, :, p * TP:(p + 1) * TP],
                        start=(cp == 0), stop=(cp == NCP - 1),
                        perf_mode=DR)

        # --- phase 3 --- (SwInterleave reverses nd partitions: partition m
        # holds natural column 127-m, so [0:H]=den-rev, [H:2H]=num-rev and
        # csv is reversed to match via the antiI matmul)
        csv = cpool.tile([128, BPC], f32, name="csv", tag="csv")
        csr = cpool.tile([128, BPC], f32, name="csr", tag="csr")
        tmps = {}

        def ph3_reads(p, b):
            ncp = tpool.tile([H, TP], f32, name="ncp", tag="ncp")
            nc.scalar.activation(ncp[:, :], nd[p][b][H:2 * H, :], Identity,
                                 bias=csv[H:2 * H, b:b + 1])
            dcp = tpool.tile([H, TP], f32, name="dcp", tag="dcp")
            nc.scalar.activation(dcp[:, :], nd[p][b][0:H, :], Identity,
                                 bias=csv[0:H, b:b + 1])
            rec = tpool.tile([H, TP], f32, name="rec", tag="rec")
            nc.vector.reciprocal_approx_fast(rec[:, :], dcp[:, :])
            tmp = tpool.tile([H, TP], f32, name="tmp", tag="tmp")
            nc.vector.tensor_tensor(tmp[:, :], ncp[:, :], rec[:, :], mult)
            tmps[(p, b)] = tmp

        def ph3_tail(p, b):
            yt = ytpool.tile([H + 1, TP], bf16, name="yt", tag="yt")
            nc.gpsimd.memset(yt[H:H + 1, :], 1.0)
            nc.gpsimd.tensor_tensor(
                yt[0:H, :], tmps[(p, b)][:, :],
                sq_sb[b][:, p * TP:(p + 1) * TP], mult)
            for c in range(4):
                po = pps.tile([128, 512], f32, name="po", tag="ps")
                nc.tensor.matmul(po[:, :], lhsT=yt[:, c * 128:(c + 1) * 128],
                                 rhs=wp_sb[:, :], start=True, stop=True)
                osb = opool.tile([128, 512], bf16, name="osb", tag="osb")
                if c % 2 == 0:
                    nc.scalar.copy(osb[:, :], po[:, :])
                else:
                    nc.vector.tensor_copy(osb[:, :], po[:, :])
                t0 = p * TP + c * 128
                eng = (nc.gpsimd, nc.sync)[c % 2]
                eng.dma_start(out[b, t0:t0 + 128, :], osb[:, :])

        # --- emission schedule (PE order = program order per engine) ---
        ph1_group(0)
        ph1_group(1)
        q_block(0)
        q_block(1)
        ph1_group(2)
        nd_alloc((0, 1))
        for cp in range(0, 4):
            ph2_cp(cp, (0, 1))
        ph1_group(3)
        for b in range(BPC):
            nc.scalar.copy(csr[:, b:b + 1], cs_ps[b][:, 0:1])
            pcs_r = pps.tile([128, 512], f32, name="pcsr", tag="ps")
            nc.tensor.matmul(pcs_r[:, 0:1], lhsT=antiI_sb[:, :],
                             rhs=csr[:, b:b + 1], start=True, stop=True)
            nc.scalar.copy(csv[:, b:b + 1], pcs_r[:, 0:1])
        for cp in range(4, NCP):
            ph2_cp(cp, (0, 1))
        for p in (0, 1):
            for b in range(BPC):
                ph3_reads(p, b)
        nd_alloc((2,))
        for cp in range(0, 4):
            ph2_cp(cp, (2,))
        ph3_tail(0, 0)
        ph3_tail(0, 1)
        for cp in range(4, 8):
            ph2_cp(cp, (2,))
        ph3_tail(1, 0)
        ph3_tail(1, 1)
        for cp in range(8, NCP):
            ph2_cp(cp, (2,))
        for b in range(BPC):
            ph3_reads(2, b)
        nd_alloc((3,))
        for cp in range(0, 4):
            ph2_cp(cp, (3,))
        ph3_tail(2, 0)
        for cp in range(4, 8):
            ph2_cp(cp, (3,))
        ph3_tail(2, 1)
        for cp in range(8, NCP):
            ph2_cp(cp, (3,))
        for b in range(BPC):
            ph3_reads(3, b)
        ph3_tail(3, 0)
        ph3_tail(3, 1)

    nc.compile()
    from concourse.bass_interp import get_hw_module
    nc.m = get_hw_module(nc.m)
    return nc


def _get_module(use_bias):
    key = ("nc", use_bias)
    if key not in _NC_CACHE:
        _NC_CACHE[key] = _build_module(use_bias)
    return _NC_CACHE[key]


def kernel(x, wq, bq, wk, bk, wv, bv, wp, bp, wbias):
    from concourse.bass_utils import run_bass_kernel_spmd

    bf16 = ml_dtypes.bfloat16
    f8 = ml_dtypes.float8_e4m3
    x = np.asarray(x, np.float32)
    wbias = np.asarray(wbias, np.float32)
    wq, wk, wv, wp = (np.asarray(a, np.float32) for a in (wq, wk, wv, wp))
    bq, bk, bv, bp = (np.asarray(a, np.float32) for a in (bq, bk, bv, bp))

    xT_full = np.ascontiguousarray(x.transpose(0, 2, 1)).astype(bf16)
    rq_full = (SCALE * (np.exp(wbias) - 1.0)).T     # [s, t] f32

    # pack the d-chunked SBUF layouts host-side
    wkv_h = np.ascontiguousarray(
        np.concatenate([wk.T, wv.T], axis=1).astype(bf16)
        .reshape(4, 128, 2 * H).transpose(1, 0, 2).reshape(128, 4 * 2 * H))
    # h-reversed Q/projection weights: the SwInterleave phase-2 matmul emits
    # nd with reversed partitions, so sq/tmp/yt all live in reversed-h order
    wqT_h = np.ascontiguousarray(
        wq.T[:, ::-1].astype(bf16).reshape(4, 128, H).transpose(1, 0, 2)
        .reshape(128, 4 * H))
    wpT_h = np.concatenate(
        [wp.T[::-1], np.asarray(bp, np.float32)[None, :]],
        axis=0).astype(bf16)
    bkv_h = np.concatenate([bk, bv])[None, :].astype(bf16)         # [1, 2H]
    bq_h = np.asarray(bq, np.float32)[::-1, None].copy()           # [H, 1]
    ones_h = np.ones((1, 128), dtype=bf16)
    antiI_h = np.eye(128, dtype=np.float32)[:, ::-1].copy()        # [128,128]
    use_bias = bool(np.any(bk) or np.any(bv))

    # Per t-group: s-permuted inputs (own t-slice rows first) so the SPMD
    # graph reads Q's x columns at [0:TPC] on every core.
    perm = {}
    for tj in range(TG):
        perm[tj] = np.concatenate([
            np.arange(tj * TPC, (tj + 1) * TPC),
            np.arange(0, tj * TPC),
            np.arange((tj + 1) * TPC, T)])

    # r chunk-pair layout for DoubleRow*: rp[cp][p, i*TPC + t] =
    # r[s = 256*cp + 128*i + p, t]
    rp_tj = {}
    for tj in range(TG):
        rq = rq_full[perm[tj]][:, tj * TPC:(tj + 1) * TPC].astype(f8)
        rp_tj[tj] = np.ascontiguousarray(
            rq.reshape(NCP, 2, 128, TPC).transpose(0, 2, 1, 3)
            .reshape(NCP, 128, 2 * TPC))

    xp_c = {}
    for bi in range(BG):
        for tj in range(TG):
            xt = xT_full[bi * BPC:(bi + 1) * BPC][:, :, perm[tj]]
            xp_c[(bi, tj)] = np.ascontiguousarray(
                xt.reshape(BPC, 4, 128, NG, 1024).transpose(0, 3, 2, 1, 4)
                .reshape(BPC, NG, 128, 4096))

    in_maps = []
    for c in range(N_CORES):
        bi, tj = c // TG, c % TG
        in_maps.append({
            "xp": xp_c[(bi, tj)],
            "rp": rp_tj[tj],
            "wkv": wkv_h, "wqT": wqT_h, "wpT": wpT_h,
            "bkv": bkv_h, "bqv": bq_h, "ones": ones_h, "antiI": antiI_h,
        })

    nc = _get_module(use_bias)
    res = run_bass_kernel_spmd(nc, in_maps, core_ids=list(range(N_CORES)))

    full = np.empty((B, T, D), dtype=np.float32)
    for c in range(N_CORES):
        bi, tj = c // TG, c % TG
        full[bi * BPC:(bi + 1) * BPC, tj * TPC:(tj + 1) * TPC, :] = \
            res.results[c]["out"].astype(np.float32)
    return full
